# revision 1
# baseline (speedup 1.0000x reference)
"""Causal FFT-conv (B=32, Cin=Cout=128, L=K=4096) for 8 trn2 NeuronCores.

Strategy: host computes rFFTs (N=8192) of padded x and conj-rFFT of the
weight; the dominant frequency-domain channel contraction
  yhat[f, b, o] = sum_c xhat[f, c, b] * ghat[f, c, o]   (complex)
runs on-device as fp32 matmuls, sharded over frequency across the 8
cores (each frequency bin is independent).  Host then does the inverse
rFFT, crops to L, and adds bias.
"""

import sys

sys.path.insert(0, "/opt/trn_rl_repo")

import numpy as np

B, C, O, L, K = 32, 128, 128, 4096, 4096
N = 8192            # linear-conv FFT length (L + (K+1) - 1 with odd-padded kernel)
F = N // 2 + 1      # 4097 rfft bins
NCORES = 8
last_exec_ns = None
_nc_cache = None
FC = 513            # per-core frequency bins (8*513 = 4104 >= 4097, zero padded)
NFB = 19            # f-block per inner loop iteration; 27 blocks of 19 = 513
assert NFB * 27 == FC


def _build_bass():
    from concourse import bass, bacc, mybir
    from concourse.tile import TileContext

    dt = mybir.dt.float32
    dtb = mybir.dt.bfloat16
    nc = bacc.Bacc(None, target_bir_lowering=False)
    # Per-core inputs, frequency-major so the c-contraction is the partition dim.
    # per-f row layout along free dim: [ReX(32) | ImX(32) | -ImX(32) | ReW(128) | ImW(128)]
    pk = nc.dram_tensor("pk", [FC, C, 3 * B + 2 * O], dtb, kind="ExternalInput")
    y = nc.dram_tensor("y", [FC, 2, B, O], dt, kind="ExternalOutput")

    with TileContext(nc) as tc:
        with (
            tc.tile_pool(name="xin", bufs=3) as xpool,
            tc.tile_pool(name="yout", bufs=6) as ypool,
            tc.tile_pool(name="ps", bufs=4, space="PSUM") as pspool,
        ):
            for blk in range(FC // NFB):
                f0 = blk * NFB
                W = 3 * B + 2 * O
                xt = xpool.tile([C, NFB * W], dtb, tag="x")
                nc.gpsimd.dma_start(out=xt.rearrange("c (f z) -> c f z", f=NFB), in_=pk[f0 : f0 + NFB].rearrange("f c z -> c f z"))
                # 27 bins -> 7 psum-bank groups of <=4 bins (4*128 fp32 = 1 bank)
                for g0 in range(0, NFB, 4):
                    gn = min(4, NFB - g0)
                    yr = pspool.tile([B, gn * O], dt, tag="yr")
                    yi = pspool.tile([B, gn * O], dt, tag="yi")
                    def ops(fi):
                        xb = fi * W
                        A = xt[:, xb : xb + B]
                        Bt = xt[:, xb + B : xb + 2 * B]
                        Bn = xt[:, xb + 2 * B : xb + 3 * B]
                        Cc = xt[:, xb + 3 * B : xb + 3 * B + O]
                        Dd = xt[:, xb + 3 * B + O : xb + 3 * B + 2 * O]
                        return A, Bt, Bn, Cc, Dd
                    # one accumulation group per PSUM bank (waits stay small)
                    for j in range(gn):
                        A, Bt, Bn, Cc, Dd = ops(g0 + j)
                        o_sl = slice(j * O, (j + 1) * O)
                        nc.tensor.matmul(yr[:, o_sl], A, Cc, start=(j == 0), stop=False)
                        nc.tensor.matmul(yr[:, o_sl], Bn, Dd, start=False, stop=(j == gn - 1))
                    for j in range(gn):
                        A, Bt, Bn, Cc, Dd = ops(g0 + j)
                        o_sl = slice(j * O, (j + 1) * O)
                        nc.tensor.matmul(yi[:, o_sl], A, Dd, start=(j == 0), stop=False)
                        nc.tensor.matmul(yi[:, o_sl], Bt, Cc, start=False, stop=(j == gn - 1))
                    yrs = ypool.tile([B, gn * O], dt, tag="yrs")
                    yis = ypool.tile([B, gn * O], dt, tag="yis")
                    nc.vector.tensor_copy(yrs, yr)
                    nc.vector.tensor_copy(yis, yi)
                    nc.gpsimd.dma_start(
                        out=y[f0 + g0 : f0 + g0 + gn, 0].rearrange("f b o -> b f o"),
                        in_=yrs.rearrange("b (f o) -> b f o", f=gn),
                    )
                    nc.gpsimd.dma_start(
                        out=y[f0 + g0 : f0 + g0 + gn, 1].rearrange("f b o -> b f o"),
                        in_=yis.rearrange("b (f o) -> b f o", f=gn),
                    )
    nc.compile()
    return nc


def kernel(x: np.ndarray, weight: np.ndarray, bias: np.ndarray) -> np.ndarray:
    from concourse.bass_utils import run_bass_kernel_spmd

    x = np.asarray(x, np.float32)
    weight = np.asarray(weight, np.float32)
    bias = np.asarray(bias, np.float32)

    # Host FFTs (match reference: odd-pad kernel left by 1, causal left-pad x).
    xp = np.pad(x, ((0, 0), (0, 0), (4096, 0)))                  # [B, C, 8192]
    wp = np.pad(weight, ((0, 0), (0, 0), (1, 0)))                # [O, C, 4097]
    xf = np.fft.rfft(xp, axis=-1)                                # [B, C, F]
    gf = np.conj(np.fft.rfft(wp, n=N, axis=-1))                  # [O, C, F]

    # Pad F -> 8*FC and reshape to per-core frequency slices, f-major.
    FP = NCORES * FC
    xfp = np.zeros((B, C, FP), np.complex64)
    xfp[:, :, :F] = xf
    gfp = np.zeros((O, C, FP), np.complex64)
    gfp[:, :, :F] = gf
    xfp = np.ascontiguousarray(xfp.transpose(2, 1, 0))           # [FP, C, B]
    gfp = np.ascontiguousarray(gfp.transpose(2, 1, 0))           # [FP, C, O]

    in_maps = []
    for r in range(NCORES):
        sl = slice(r * FC, (r + 1) * FC)
        xs, gs = xfp[sl], gfp[sl]
        pk = np.concatenate(
            [xs.real, xs.imag, -xs.imag, gs.real, gs.imag], axis=2
        )  # [FC, C, 3B+2O]
        import ml_dtypes
        in_maps.append({"pk": np.ascontiguousarray(pk).astype(ml_dtypes.bfloat16)})

    global _nc_cache
    if _nc_cache is None:
        _nc_cache = _build_bass()
    nc = _nc_cache
    res = run_bass_kernel_spmd(nc, in_maps, list(range(NCORES)))
    global last_exec_ns
    last_exec_ns = getattr(res, "exec_time_ns", None)
    ys = [res.results[r]["y"] for r in range(NCORES)]            # [FC, 2, B, O]
    yall = np.concatenate(ys, axis=0)[:F]                        # [F, 2, B, O]
    yf = (yall[:, 0] + 1j * yall[:, 1]).transpose(1, 2, 0)       # [B, O, F]
    out = np.fft.irfft(yf, n=N, axis=-1)[:, :, :L].astype(np.float32)
    return out + bias[None, :, None].astype(np.float32)



# revision 2
# speedup vs baseline: 14.8327x; 14.8327x over previous
"""Causal FFT-conv (B=32, Cin=Cout=128, L=K=4096) on 8 trn2 NeuronCores.

out = conv1d(x, w, causal) computed as
  out = irfft( rfft(x,8192) . conj(rfft(w,8192)) )[(l+4097) mod 8192], l<4096
(no explicit padding anywhere: the reference's causal left-pad of x by 4096
and the odd-length weight left-pad by 1 reduce to a circular output shift).

The frequency-domain channel contraction runs on-device, sharded over
frequency bins (512 bins/core; bin 4096 handled on host). Per bin f:
  Y[o, 2k]   = sum_c Wr[c,o] V[c,2k]  + Wi[c,o] nV[c,2k]
  Y[o, 2k+1] = sum_c Wr[c,o] V[c,2k+1]+ Wi[c,o] nV[c,2k+1]
where V = complex-interleaved X over batch (pairs [Xr(b), Xi(b)]) and
nV = view(-i X) (pairs [Xi(b), -Xr(b)]): two matmuls per bin
(lhsT=Wr/Wi stationary [c,128], rhs=V/nV moving [c,64]) produce Y[o,:]
directly in complex64 memory layout — no host shuffles on the output.

Device kernel: For_i hardware loop (16 iters x 32 bins), PSUM [128,512]
per 8 bins, DVE copy (f32->bf16) to SBUF, DMA out. The
jit(shard_map(bass_exec)) callable is built once at import and cached;
per-core shards are device_put right after packing; output fetched
per-shard, pipelined.
"""

import os
import sys
import time

sys.path.insert(0, "/opt/trn_rl_repo")

import numpy as np
import scipy.fft as sfft
import ml_dtypes

BF16 = ml_dtypes.bfloat16

B, C, O, L, K = 32, 128, 128, 4096, 4096
N = 8192
F = N // 2 + 1      # 4097
NCORES = 8
FC = 512            # frequency bins per core on device (8*512 = 4096)
FB = 32             # bins per For_i iteration
NITER = FC // FB    # 16
XW = 4 * B          # 128 cols/bin in xk: [V(64) | -iV(64)]
GW = 2 * O          # 256 cols/bin in gk: [Wr(128) | Wi(128)]
YW = 2 * B          # 64 cols/bin in y: interleaved (re,im) over b

last_exec_ns = None
_runner = None

_DEV_TIMING = bool(os.environ.get("KV_TIMING"))


def _tlog(msg):
    if _DEV_TIMING:
        print(f"[kv] {msg}", file=sys.stderr, flush=True)


def _build_bass():
    from concourse import bacc, mybir
    from concourse.bass import ts
    from concourse.tile import TileContext

    dt = mybir.dt.float32
    dtb = mybir.dt.bfloat16
    nc = bacc.Bacc(None, target_bir_lowering=False)
    xk = nc.dram_tensor("xk", [C, FC * XW], dtb, kind="ExternalInput")
    gk = nc.dram_tensor("gk", [C, FC * GW], dtb, kind="ExternalInput")
    y = nc.dram_tensor("y", [O, FC * YW], dtb, kind="ExternalOutput")

    with TileContext(nc) as tc:
        with (
            tc.tile_pool(name="xin", bufs=3) as xpool,
            tc.tile_pool(name="gin", bufs=3) as gpool,
            tc.tile_pool(name="yout", bufs=3) as ypool,
            tc.tile_pool(name="ps", bufs=8, space="PSUM") as pspool,
        ):
            with tc.For_i(0, NITER, 1) as it:
                xt = xpool.tile([C, FB * XW], dtb, tag="x")
                gt = gpool.tile([C, FB * GW], dtb, tag="g")
                nc.gpsimd.dma_start(out=xt, in_=xk[:, ts(it, FB * XW)])
                nc.gpsimd.dma_start(out=gt, in_=gk[:, ts(it, FB * GW)])
                yo = ypool.tile([O, FB * YW], dtb, tag="y")
                for g in range(FB // 8):
                    ps = pspool.tile([O, 8 * YW], dt, tag="ps")  # one PSUM bank
                    for j in range(8):
                        k = g * 8 + j
                        V = xt[:, k * XW : k * XW + 2 * B]
                        nV = xt[:, k * XW + 2 * B : k * XW + 4 * B]
                        Wr = gt[:, k * GW : k * GW + O]
                        Wi = gt[:, k * GW + O : k * GW + 2 * O]
                        o_sl = ps[:, j * YW : (j + 1) * YW]
                        nc.tensor.matmul(o_sl, Wr, V, start=(j == 0), stop=False)
                        nc.tensor.matmul(o_sl, Wi, nV, start=False, stop=(j == 7))
                    nc.vector.tensor_copy(yo[:, g * 8 * YW : (g + 1) * 8 * YW], ps)
                nc.gpsimd.dma_start(out=y[:, ts(it, FB * YW)], in_=yo)
    nc.compile()
    return nc


class _Runner:
    """Builds the jit(shard_map(bass_exec)) once; reuses it per call."""

    def __init__(self):
        import jax
        import jax.numpy as jnp
        from jax.sharding import Mesh, NamedSharding, PartitionSpec
        from jax.experimental.shard_map import shard_map
        from concourse import bass2jax, mybir

        t0 = time.time()
        self.jax = jax
        nc = _build_bass()
        self.nc = nc
        _tlog(f"build_bass: {time.time()-t0:.2f} s")

        bass2jax.install_neuronx_cc_hook()

        partition_name = (
            nc.partition_id_tensor.name if nc.partition_id_tensor else None
        )
        in_names, out_names, out_avals = [], [], []
        self.extra_inputs = {}  # name -> np zeros (e.g. dbg_addr)
        for alloc in nc.m.functions[0].allocations:
            if not isinstance(alloc, mybir.MemoryLocationSet):
                continue
            name = alloc.memorylocations[0].name
            if alloc.kind == "ExternalInput":
                if name != partition_name:
                    in_names.append(name)
                    if name not in ("xk", "gk"):
                        if nc.dbg_addr is not None and name == nc.dbg_addr.name:
                            self.extra_inputs[name] = np.zeros((1, 2), np.uint32)
                        else:
                            self.extra_inputs[name] = np.zeros(
                                tuple(alloc.tensor_shape), mybir.dt.np(alloc.dtype)
                            )
            elif alloc.kind == "ExternalOutput":
                out_names.append(name)
                out_avals.append(
                    jax.core.ShapedArray(
                        tuple(alloc.tensor_shape), mybir.dt.np(alloc.dtype)
                    )
                )
        assert out_names == ["y"], out_names
        assert in_names[:2] == ["xk", "gk"], in_names
        n_params = len(in_names)
        all_in = list(in_names) + list(out_names)
        if partition_name is not None:
            all_in.append(partition_name)
        donate = tuple(range(n_params, n_params + len(out_names)))

        def _body(*args):
            operands = list(args)
            if partition_name is not None:
                operands.append(bass2jax.partition_id_tensor())
            outs = bass2jax._bass_exec_p.bind(
                *operands,
                out_avals=tuple(out_avals),
                in_names=tuple(all_in),
                out_names=tuple(out_names),
                lowering_input_output_aliases=(),
                sim_require_finite=True,
                sim_require_nnan=True,
                nc=nc,
            )
            return tuple(outs)

        devices = jax.devices()[:NCORES]
        assert len(devices) == NCORES
        self.devices = devices
        mesh = Mesh(np.asarray(devices), ("core",))
        self.sharding = NamedSharding(mesh, PartitionSpec("core"))
        in_specs = (PartitionSpec("core"),) * (n_params + len(out_names))
        out_specs = (PartitionSpec("core"),) * len(out_names)
        self.sharded = jax.jit(
            shard_map(
                _body,
                mesh=mesh,
                in_specs=in_specs,
                out_specs=out_specs,
                check_rep=False,
            ),
            donate_argnums=donate,
            keep_unused=True,
        )
        sh = self.sharding
        self.zeros_y = jax.jit(
            lambda: jnp.zeros((NCORES * O, FC * YW), jnp.bfloat16), out_shardings=sh
        )
        self.zeros_x = jax.jit(
            lambda: jnp.zeros((NCORES * C, FC * XW), jnp.bfloat16), out_shardings=sh
        )
        self.zeros_g = jax.jit(
            lambda: jnp.zeros((NCORES * C, FC * GW), jnp.bfloat16), out_shardings=sh
        )
        # Warm up: trace + neuronxcc compile + device init with
        # device-resident zeros (no tunnel traffic).
        t0 = time.time()
        args = [self.zeros_x(), self.zeros_g()]
        args += [self._rep(z) for z in self.extra_inputs.values()]
        args.append(self.zeros_y())
        outs = self.sharded(*args)
        jax.block_until_ready(outs)
        _tlog(f"warmup jit+compile: {time.time()-t0:.2f} s")

    def _rep(self, z):
        jax = self.jax
        big = np.concatenate([z] * NCORES, axis=0)
        return jax.device_put(big, self.sharding)

    def put_shards(self, xks, gks):
        jax = self.jax
        sx = [jax.device_put(xks[r], self.devices[r]) for r in range(NCORES)]
        sg = [jax.device_put(gks[r], self.devices[r]) for r in range(NCORES)]
        return sx, sg

    def run_shards(self, sx, sg):
        jax = self.jax
        t0 = time.time()
        gx = jax.make_array_from_single_device_arrays(
            (NCORES * C, FC * XW), self.sharding, sx
        )
        gg = jax.make_array_from_single_device_arrays(
            (NCORES * C, FC * GW), self.sharding, sg
        )
        args = [gx, gg]
        args += [self._rep(z) for z in self.extra_inputs.values()]
        args.append(self.zeros_y())
        outs = self.sharded(*args)
        jax.block_until_ready(outs)
        t1 = time.time()
        shards = sorted(
            outs[0].addressable_shards, key=lambda s: s.index[0].start or 0
        )
        datas = [s.data for s in shards]
        for d in datas:
            d.copy_to_host_async()
        ys = [np.asarray(d) for d in datas]  # each (O, FC*YW) bf16
        t2 = time.time()
        _tlog(f"  put-wait+exec: {t1-t0:.2f} s  fetch: {t2-t1:.2f} s")
        return ys


def _get_runner():
    global _runner
    if _runner is None:
        _runner = _Runner()
    return _runner


def _pack(x, weight):
    """Host FFTs + packing into per-core device layouts (c-major)."""
    t0 = time.time()
    Xf = sfft.rfft(x, n=N, axis=-1)  # (B, C, 4097) complex64

    xks = [np.empty((C, FC * XW), BF16) for _ in range(NCORES)]
    gks = [np.empty((C, FC * GW), BF16) for _ in range(NCORES)]
    w4096 = np.empty((C, O), np.float32)
    x4096 = np.ascontiguousarray(Xf[:, :, 4096].real.T)  # (C, B)

    nVv = np.empty((4096, 2 * B), np.float32)
    xrow = np.empty((4096, XW), BF16)
    grow = np.empty((4096, GW), BF16)
    WcT = np.empty((4097, O), np.complex64)
    for c in range(C):
        A = np.ascontiguousarray(Xf[:, c, :4096].T)  # (4096, 32) c64
        Av = A.view(np.float32)                      # (4096, 64) re/im pairs
        nVv[:, 0::2] = Av[:, 1::2]
        nVv[:, 1::2] = -Av[:, 0::2]
        xrow[:, : 2 * B] = Av
        xrow[:, 2 * B :] = nVv
        xr = xrow.reshape(NCORES, FC * XW)
        Wc = sfft.rfft(weight[:, c, :], n=N, axis=-1)  # (O=128, 4097) c64
        w4096[c] = Wc[:, 4096].real
        WcT[:] = Wc.T  # complex transpose (sequential writes)
        Wv = WcT.view(np.float32)  # (4097, 256) re/im interleaved over o
        grow[:, :O] = Wv[:4096, 0::2]
        grow[:, O:] = Wv[:4096, 1::2]
        gr = grow.reshape(NCORES, FC * GW)
        for r in range(NCORES):
            xks[r][c] = xr[r]
            gks[r][c] = gr[r]
    _tlog(f"pack: {time.time()-t0:.2f} s")
    return xks, gks, x4096, w4096


def kernel(x: np.ndarray, weight: np.ndarray, bias: np.ndarray) -> np.ndarray:
    x = np.ascontiguousarray(x, np.float32)
    weight = np.ascontiguousarray(weight, np.float32)
    bias = np.asarray(bias, np.float32)

    runner = _get_runner()
    xks, gks, x4096, w4096 = _pack(x, weight)
    t2 = time.time()
    sx, sg = runner.put_shards(xks, gks)
    ys = runner.run_shards(sx, sg)
    t3 = time.time()
    _tlog(f"spmd run: {t3-t2:.2f} s")

    Yfull = np.empty((O, F, B), np.complex64)
    Yv = Yfull.view(np.float32).reshape(O, F, 2 * B)
    for r in range(NCORES):
        Yv[:, FC * r : FC * (r + 1), :] = ys[r].reshape(O, FC, YW)
    Yfull[:, 4096, :] = (w4096.T @ x4096).astype(np.complex64)
    yt = sfft.irfft(Yfull, n=N, axis=1)  # (O, 8192, B) f32
    out = np.empty((B, O, L), np.float32)
    out[:, :, : L - 1] = yt[:, 4097:8192, :].transpose(2, 0, 1)
    out[:, :, L - 1] = yt[:, 0, :].T
    out += bias[None, :, None]
    _tlog(f"post: {time.time()-t3:.2f} s")
    return out


if not os.environ.get("KV_NO_EAGER"):
    try:
        _get_runner()
    except Exception as e:  # defer to first call if devices unavailable now
        _tlog(f"eager init failed ({e!r}); will retry lazily")
        _runner = None


# revision 3
# speedup vs baseline: 27.6792x; 1.8661x over previous
"""Causal FFT-conv (B=32, Cin=Cout=128, L=K=4096) on 8 trn2 NeuronCores.

out = conv1d(x, w, causal) computed as
  out = irfft( rfft(x,8192) . conj(rfft(w,8192)) )[(l+4097) mod 8192], l<4096
(no explicit padding anywhere: the reference's causal left-pad of x by 4096
and the odd-length weight left-pad by 1 reduce to a circular output shift).

The frequency-domain channel contraction runs on-device, sharded over
frequency bins (512 bins/core; bin 4096 handled on host). Per bin f:
  Y[o, 2k]   = sum_c Wr[c,o] V[c,2k]  + Wi[c,o] nV[c,2k]
  Y[o, 2k+1] = sum_c Wr[c,o] V[c,2k+1]+ Wi[c,o] nV[c,2k+1]
where V = complex-interleaved X over batch (pairs [Xr(b), Xi(b)]) and
nV = view(-i X) (pairs [Xi(b), -Xr(b)]): two matmuls per bin
(lhsT=Wr/Wi stationary [c,128], rhs=V/nV moving [c,64]) produce Y[o,:]
directly in complex64 memory layout — no host shuffles on the output.

Device kernel: For_i hardware loop (16 iters x 32 bins), PSUM [128,512]
per 8 bins, DVE copy (f32->bf16) to SBUF, DMA out. The
jit(shard_map(bass_exec)) callable is built once at import and cached;
per-core shards are device_put right after packing; output fetched
per-shard, pipelined.
"""

import os
import sys
import time

sys.path.insert(0, "/opt/trn_rl_repo")

import numpy as np
import scipy.fft as sfft
import ml_dtypes

BF16 = ml_dtypes.bfloat16

B, C, O, L, K = 32, 128, 128, 4096, 4096
N = 8192
F = N // 2 + 1      # 4097
NCORES = 8
FC = 512            # frequency bins per core on device (8*512 = 4096)
FB = 32             # bins per For_i iteration
NITER = FC // FB    # 16
XW = 4 * B          # 128 cols/bin in xk: [V(64) | -iV(64)]
GW = 2 * O          # 256 cols/bin in gk: [Wr(128) | Wi(128)]
YW = 2 * B          # 64 cols/bin in y: interleaved (re,im) over b

last_exec_ns = None
_runner = None

_DEV_TIMING = bool(os.environ.get("KV_TIMING"))


def _tlog(msg):
    if _DEV_TIMING:
        print(f"[kv] {msg}", file=sys.stderr, flush=True)


def _build_bass():
    from concourse import bacc, mybir
    from concourse.bass import ts
    from concourse.tile import TileContext

    dt = mybir.dt.float32
    dtb = mybir.dt.bfloat16
    nc = bacc.Bacc(None, target_bir_lowering=False)
    xk = nc.dram_tensor("xk", [C, FC * XW], dtb, kind="ExternalInput")
    gk = nc.dram_tensor("gk", [C, FC * GW], dtb, kind="ExternalInput")
    y = nc.dram_tensor("y", [O, FC * YW], dtb, kind="ExternalOutput")

    with TileContext(nc) as tc:
        with (
            tc.tile_pool(name="xin", bufs=3) as xpool,
            tc.tile_pool(name="gin", bufs=3) as gpool,
            tc.tile_pool(name="yout", bufs=3) as ypool,
            tc.tile_pool(name="ps", bufs=8, space="PSUM") as pspool,
        ):
            with tc.For_i(0, NITER, 1) as it:
                xt = xpool.tile([C, FB * XW], dtb, tag="x")
                gt = gpool.tile([C, FB * GW], dtb, tag="g")
                nc.gpsimd.dma_start(out=xt, in_=xk[:, ts(it, FB * XW)])
                nc.gpsimd.dma_start(out=gt, in_=gk[:, ts(it, FB * GW)])
                yo = ypool.tile([O, FB * YW], dtb, tag="y")
                for g in range(FB // 8):
                    ps = pspool.tile([O, 8 * YW], dt, tag="ps")  # one PSUM bank
                    for j in range(8):
                        k = g * 8 + j
                        V = xt[:, k * XW : k * XW + 2 * B]
                        nV = xt[:, k * XW + 2 * B : k * XW + 4 * B]
                        Wr = gt[:, k * GW : k * GW + O]
                        Wi = gt[:, k * GW + O : k * GW + 2 * O]
                        o_sl = ps[:, j * YW : (j + 1) * YW]
                        nc.tensor.matmul(o_sl, Wr, V, start=(j == 0), stop=False)
                        nc.tensor.matmul(o_sl, Wi, nV, start=False, stop=(j == 7))
                    nc.vector.tensor_copy(yo[:, g * 8 * YW : (g + 1) * 8 * YW], ps)
                nc.gpsimd.dma_start(out=y[:, ts(it, FB * YW)], in_=yo)
    nc.compile()
    return nc


class _Runner:
    """Builds the jit(shard_map(bass_exec)) once; reuses it per call."""

    def __init__(self):
        import jax
        import jax.numpy as jnp
        from jax.sharding import Mesh, NamedSharding, PartitionSpec
        from jax.experimental.shard_map import shard_map
        from concourse import bass2jax, mybir

        t0 = time.time()
        self.jax = jax
        nc = _build_bass()
        self.nc = nc
        _tlog(f"build_bass: {time.time()-t0:.2f} s")

        bass2jax.install_neuronx_cc_hook()

        partition_name = (
            nc.partition_id_tensor.name if nc.partition_id_tensor else None
        )
        in_names, out_names, out_avals = [], [], []
        self.extra_inputs = {}  # name -> np zeros (e.g. dbg_addr)
        for alloc in nc.m.functions[0].allocations:
            if not isinstance(alloc, mybir.MemoryLocationSet):
                continue
            name = alloc.memorylocations[0].name
            if alloc.kind == "ExternalInput":
                if name != partition_name:
                    in_names.append(name)
                    if name not in ("xk", "gk"):
                        if nc.dbg_addr is not None and name == nc.dbg_addr.name:
                            self.extra_inputs[name] = np.zeros((1, 2), np.uint32)
                        else:
                            self.extra_inputs[name] = np.zeros(
                                tuple(alloc.tensor_shape), mybir.dt.np(alloc.dtype)
                            )
            elif alloc.kind == "ExternalOutput":
                out_names.append(name)
                out_avals.append(
                    jax.core.ShapedArray(
                        tuple(alloc.tensor_shape), mybir.dt.np(alloc.dtype)
                    )
                )
        assert out_names == ["y"], out_names
        assert in_names[:2] == ["xk", "gk"], in_names
        n_params = len(in_names)
        all_in = list(in_names) + list(out_names)
        if partition_name is not None:
            all_in.append(partition_name)
        donate = tuple(range(n_params, n_params + len(out_names)))

        def _body(*args):
            operands = list(args)
            if partition_name is not None:
                operands.append(bass2jax.partition_id_tensor())
            outs = bass2jax._bass_exec_p.bind(
                *operands,
                out_avals=tuple(out_avals),
                in_names=tuple(all_in),
                out_names=tuple(out_names),
                lowering_input_output_aliases=(),
                sim_require_finite=True,
                sim_require_nnan=True,
                nc=nc,
            )
            return tuple(outs)

        devices = jax.devices()[:NCORES]
        assert len(devices) == NCORES
        self.devices = devices
        mesh = Mesh(np.asarray(devices), ("core",))
        self.sharding = NamedSharding(mesh, PartitionSpec("core"))
        in_specs = (PartitionSpec("core"),) * (n_params + len(out_names))
        out_specs = (PartitionSpec("core"),) * len(out_names)
        self.sharded = jax.jit(
            shard_map(
                _body,
                mesh=mesh,
                in_specs=in_specs,
                out_specs=out_specs,
                check_rep=False,
            ),
            donate_argnums=donate,
            keep_unused=True,
        )
        sh = self.sharding
        self.zeros_y = jax.jit(
            lambda: jnp.zeros((NCORES * O, FC * YW), jnp.bfloat16), out_shardings=sh
        )
        self.zeros_x = jax.jit(
            lambda: jnp.zeros((NCORES * C, FC * XW), jnp.bfloat16), out_shardings=sh
        )
        self.zeros_g = jax.jit(
            lambda: jnp.zeros((NCORES * C, FC * GW), jnp.bfloat16), out_shardings=sh
        )
        # Warm up: trace + neuronxcc compile + device init with
        # device-resident zeros (no tunnel traffic).
        t0 = time.time()
        args = [self.zeros_x(), self.zeros_g()]
        args += [self._rep(z) for z in self.extra_inputs.values()]
        args.append(self.zeros_y())
        outs = self.sharded(*args)
        jax.block_until_ready(outs)
        _tlog(f"warmup jit+compile: {time.time()-t0:.2f} s")

    def _rep(self, z):
        jax = self.jax
        big = np.concatenate([z] * NCORES, axis=0)
        return jax.device_put(big, self.sharding)

    def put_shards(self, xks, gks):
        jax = self.jax
        sx = [jax.device_put(xks[r], self.devices[r]) for r in range(NCORES)]
        sg = [jax.device_put(gks[r], self.devices[r]) for r in range(NCORES)]
        return sx, sg

    def run_shards(self, sx, sg):
        jax = self.jax
        t0 = time.time()
        gx = jax.make_array_from_single_device_arrays(
            (NCORES * C, FC * XW), self.sharding, sx
        )
        gg = jax.make_array_from_single_device_arrays(
            (NCORES * C, FC * GW), self.sharding, sg
        )
        args = [gx, gg]
        args += [self._rep(z) for z in self.extra_inputs.values()]
        args.append(self.zeros_y())
        outs = self.sharded(*args)
        jax.block_until_ready(outs)
        t1 = time.time()
        shards = sorted(
            outs[0].addressable_shards, key=lambda s: s.index[0].start or 0
        )
        datas = [s.data for s in shards]
        for d in datas:
            d.copy_to_host_async()
        ys = [np.asarray(d) for d in datas]  # each (O, FC*YW) bf16
        t2 = time.time()
        _tlog(f"  put-wait+exec: {t1-t0:.2f} s  fetch: {t2-t1:.2f} s")
        return ys


def _get_runner():
    global _runner
    if _runner is None:
        last = None
        for attempt in range(3):
            try:
                _runner = _Runner()
                break
            except Exception as e:  # e.g. transient axon "mesh desynced"
                last = e
                _tlog(f"runner init attempt {attempt} failed: {e!r}")
                time.sleep(20)
        else:
            raise last
    return _runner


def _pack(x, weight):
    """Host FFTs + packing into per-core device layouts (c-major)."""
    t0 = time.time()
    Xf = sfft.rfft(x, n=N, axis=-1)  # (B, C, 4097) complex64

    xks = [np.empty((C, FC * XW), BF16) for _ in range(NCORES)]
    gks = [np.empty((C, FC * GW), BF16) for _ in range(NCORES)]
    w4096 = np.empty((C, O), np.float32)
    x4096 = np.ascontiguousarray(Xf[:, :, 4096].real.T)  # (C, B)

    nVv = np.empty((4096, 2 * B), np.float32)
    xrow = np.empty((4096, XW), BF16)
    grow = np.empty((4096, GW), BF16)
    WcT = np.empty((4097, O), np.complex64)
    for c in range(C):
        A = np.ascontiguousarray(Xf[:, c, :4096].T)  # (4096, 32) c64
        Av = A.view(np.float32)                      # (4096, 64) re/im pairs
        nVv[:, 0::2] = Av[:, 1::2]
        nVv[:, 1::2] = -Av[:, 0::2]
        xrow[:, : 2 * B] = Av
        xrow[:, 2 * B :] = nVv
        xr = xrow.reshape(NCORES, FC * XW)
        Wc = sfft.rfft(weight[:, c, :], n=N, axis=-1)  # (O=128, 4097) c64
        w4096[c] = Wc[:, 4096].real
        WcT[:] = Wc.T  # complex transpose (sequential writes)
        Wv = WcT.view(np.float32)  # (4097, 256) re/im interleaved over o
        grow[:, :O] = Wv[:4096, 0::2]
        grow[:, O:] = Wv[:4096, 1::2]
        gr = grow.reshape(NCORES, FC * GW)
        for r in range(NCORES):
            xks[r][c] = xr[r]
            gks[r][c] = gr[r]
    _tlog(f"pack: {time.time()-t0:.2f} s")
    return xks, gks, x4096, w4096


def kernel(x: np.ndarray, weight: np.ndarray, bias: np.ndarray) -> np.ndarray:
    x = np.ascontiguousarray(x, np.float32)
    weight = np.ascontiguousarray(weight, np.float32)
    bias = np.asarray(bias, np.float32)

    runner = _get_runner()
    xks, gks, x4096, w4096 = _pack(x, weight)
    t2 = time.time()
    sx, sg = runner.put_shards(xks, gks)
    ys = runner.run_shards(sx, sg)
    t3 = time.time()
    _tlog(f"spmd run: {t3-t2:.2f} s")

    Yfull = np.empty((O, F, B), np.complex64)
    Yv = Yfull.view(np.float32).reshape(O, F, 2 * B)
    for r in range(NCORES):
        Yv[:, FC * r : FC * (r + 1), :] = ys[r].reshape(O, FC, YW)
    Yfull[:, 4096, :] = (w4096.T @ x4096).astype(np.complex64)
    yt = sfft.irfft(Yfull, n=N, axis=1)  # (O, 8192, B) f32
    out = np.empty((B, O, L), np.float32)
    out[:, :, : L - 1] = yt[:, 4097:8192, :].transpose(2, 0, 1)
    out[:, :, L - 1] = yt[:, 0, :].T
    out += bias[None, :, None]
    _tlog(f"post: {time.time()-t3:.2f} s")
    return out


if not os.environ.get("KV_NO_EAGER"):
    try:
        _get_runner()
    except Exception as e:  # defer to first call if devices unavailable now
        _tlog(f"eager init failed ({e!r}); will retry lazily")
        _runner = None


# revision 4
# speedup vs baseline: 35.6547x; 1.2881x over previous
"""Causal FFT-conv (B=32, Cin=Cout=128, L=K=4096) on 8 trn2 NeuronCores.

out = conv1d(x, w, causal) computed as
  out = irfft( rfft(x,8192) . conj(rfft(w,8192)) )[(l+4097) mod 8192], l<4096
(no explicit padding: the reference's pads reduce to a circular output shift).

Frequency contraction on-device, sharded over bins (512/core; bin 4096 on
host). Per bin: two matmuls (lhsT=Wr/Wi stationary [c,128], rhs=V/-iV moving
[c,64]) with complex-interleaved x operands so Y lands in complex64 layout.

v6: inputs are split into 4 row-chunks along C (xk0..3 / gk0..3); the host
packs c-major and device_puts each chunk as soon as its rows are complete,
overlapping ~3/4 of the tunnel upload with packing. Output assembly is
folded into the pipelined per-shard fetch.
"""

import os
import sys
import time

sys.path.insert(0, "/opt/trn_rl_repo")

import numpy as np
import scipy.fft as sfft
import ml_dtypes

BF16 = ml_dtypes.bfloat16

B, C, O, L, K = 32, 128, 128, 4096, 4096
N = 8192
F = N // 2 + 1      # 4097
NCORES = 8
FC = 512            # frequency bins per core on device (8*512 = 4096)
FB = 32             # bins per For_i iteration
NITER = FC // FB    # 16
XW = 2 * B          # 64 cols/bin in xk: V only (re/im interleaved over b)
GW = 2 * O          # 256 cols/bin in gk: [Wr(128) | Wi(128)]
YW = 2 * B          # 64 cols/bin in y: interleaved (re,im) over b
NCH = 8             # C-chunks for upload/pack overlap
CR = C // NCH       # 32 rows per chunk

last_exec_ns = None
_runner = None

_DEV_TIMING = bool(os.environ.get("KV_TIMING"))


def _tlog(msg):
    if _DEV_TIMING:
        print(f"[kv] {msg}", file=sys.stderr, flush=True)


def _build_bass():
    from concourse import bacc, mybir
    from concourse.bass import ts
    from concourse.tile import TileContext

    dt = mybir.dt.float32
    dtb = mybir.dt.bfloat16
    nc = bacc.Bacc(None, target_bir_lowering=False)
    xks = [
        nc.dram_tensor(f"xk{q}", [CR, FC * XW], dtb, kind="ExternalInput")
        for q in range(NCH)
    ]
    gks = [
        nc.dram_tensor(f"gk{q}", [CR, FC * GW], dtb, kind="ExternalInput")
        for q in range(NCH)
    ]
    y = nc.dram_tensor("y", [O, FC * YW], dtb, kind="ExternalOutput")

    with TileContext(nc) as tc:
        with (
            tc.tile_pool(name="xin", bufs=3) as xpool,
            tc.tile_pool(name="gin", bufs=3) as gpool,
            tc.tile_pool(name="yout", bufs=3) as ypool,
            tc.tile_pool(name="ps", bufs=8, space="PSUM") as pspool,
        ):
            with tc.For_i(0, NITER, 1) as it:
                xt = xpool.tile([C, FB * XW], dtb, tag="x")
                gt = gpool.tile([C, FB * GW], dtb, tag="g")
                for q in range(NCH):
                    nc.gpsimd.dma_start(
                        out=xt[CR * q : CR * (q + 1), :],
                        in_=xks[q][:, ts(it, FB * XW)],
                    )
                    nc.gpsimd.dma_start(
                        out=gt[CR * q : CR * (q + 1), :],
                        in_=gks[q][:, ts(it, FB * GW)],
                    )
                # negated Wi blocks (block-strided 3D AP, 128-contig runs)
                nwt = gpool.tile([C, FB * O], dtb, tag="nw")
                nc.vector.tensor_scalar_mul(
                    nwt.rearrange("c (f o) -> c f o", o=O),
                    gt.rearrange("c (f w) -> c f w", w=GW)[:, :, O : 2 * O],
                    -1.0,
                )
                yo = ypool.tile([O, FB * YW], dtb, tag="y")
                for g in range(FB // 8):
                    ps = pspool.tile([O, 8 * YW], dt, tag="ps")  # one PSUM bank
                    for j in range(8):
                        k = g * 8 + j
                        V = xt[:, k * XW : (k + 1) * XW]
                        Vodd = xt[:, k * XW + 1 : (k + 1) * XW : 2]   # Xi cols
                        Vev = xt[:, k * XW : (k + 1) * XW : 2]        # Xr cols
                        Wr = gt[:, k * GW : k * GW + O]
                        Wi = gt[:, k * GW + O : k * GW + 2 * O]
                        nWi = nwt[:, k * O : (k + 1) * O]
                        o_sl = ps[:, j * YW : (j + 1) * YW]
                        o_ev = ps[:, j * YW : (j + 1) * YW : 2]
                        o_od = ps[:, j * YW + 1 : (j + 1) * YW : 2]
                        nc.tensor.matmul(o_sl, Wr, V, start=(j == 0), stop=False)
                        nc.tensor.matmul(o_ev, Wi, Vodd, start=False, stop=False)
                        nc.tensor.matmul(o_od, nWi, Vev, start=False, stop=(j == 7))
                    nc.vector.tensor_copy(yo[:, g * 8 * YW : (g + 1) * 8 * YW], ps)
                nc.gpsimd.dma_start(out=y[:, ts(it, FB * YW)], in_=yo)
    nc.compile()
    return nc


class _Runner:
    """Builds the jit(shard_map(bass_exec)) once; reuses it per call."""

    def __init__(self):
        import jax
        import jax.numpy as jnp
        from jax.sharding import Mesh, NamedSharding, PartitionSpec
        from jax.experimental.shard_map import shard_map
        from concourse import bass2jax, mybir

        t0 = time.time()
        self.jax = jax
        nc = _build_bass()
        self.nc = nc
        _tlog(f"build_bass: {time.time()-t0:.2f} s")

        bass2jax.install_neuronx_cc_hook()

        partition_name = (
            nc.partition_id_tensor.name if nc.partition_id_tensor else None
        )
        self.chunk_names = [f"xk{q}" for q in range(NCH)] + [
            f"gk{q}" for q in range(NCH)
        ]
        in_names, out_names, out_avals = [], [], []
        self.extra_inputs = {}  # name -> np zeros (e.g. dbg_addr)
        for alloc in nc.m.functions[0].allocations:
            if not isinstance(alloc, mybir.MemoryLocationSet):
                continue
            name = alloc.memorylocations[0].name
            if alloc.kind == "ExternalInput":
                if name != partition_name:
                    in_names.append(name)
                    if name not in self.chunk_names:
                        if nc.dbg_addr is not None and name == nc.dbg_addr.name:
                            self.extra_inputs[name] = np.zeros((1, 2), np.uint32)
                        else:
                            self.extra_inputs[name] = np.zeros(
                                tuple(alloc.tensor_shape), mybir.dt.np(alloc.dtype)
                            )
            elif alloc.kind == "ExternalOutput":
                out_names.append(name)
                out_avals.append(
                    jax.core.ShapedArray(
                        tuple(alloc.tensor_shape), mybir.dt.np(alloc.dtype)
                    )
                )
        assert out_names == ["y"], out_names
        assert in_names[: 2 * NCH] == self.chunk_names, in_names
        n_params = len(in_names)
        all_in = list(in_names) + list(out_names)
        if partition_name is not None:
            all_in.append(partition_name)
        donate = tuple(range(n_params, n_params + len(out_names)))

        def _body(*args):
            operands = list(args)
            if partition_name is not None:
                operands.append(bass2jax.partition_id_tensor())
            outs = bass2jax._bass_exec_p.bind(
                *operands,
                out_avals=tuple(out_avals),
                in_names=tuple(all_in),
                out_names=tuple(out_names),
                lowering_input_output_aliases=(),
                sim_require_finite=True,
                sim_require_nnan=True,
                nc=nc,
            )
            return tuple(outs)

        devices = jax.devices()[:NCORES]
        assert len(devices) == NCORES
        self.devices = devices
        mesh = Mesh(np.asarray(devices), ("core",))
        self.sharding = NamedSharding(mesh, PartitionSpec("core"))
        in_specs = (PartitionSpec("core"),) * (n_params + len(out_names))
        out_specs = (PartitionSpec("core"),) * len(out_names)
        self.sharded = jax.jit(
            shard_map(
                _body,
                mesh=mesh,
                in_specs=in_specs,
                out_specs=out_specs,
                check_rep=False,
            ),
            donate_argnums=donate,
            keep_unused=True,
        )
        sh = self.sharding

        def _zeros_all():
            xs = tuple(
                jnp.zeros((NCORES * CR, FC * XW), jnp.bfloat16) for _ in range(NCH)
            )
            gs = tuple(
                jnp.zeros((NCORES * CR, FC * GW), jnp.bfloat16) for _ in range(NCH)
            )
            return xs + gs

        self.zeros_in = jax.jit(_zeros_all, out_shardings=(sh,) * (2 * NCH))
        self.zeros_y = jax.jit(
            lambda: jnp.zeros((NCORES * O, FC * YW), jnp.bfloat16), out_shardings=sh
        )
        # Warm up: trace + neuronxcc compile + device init with
        # device-resident zeros (no tunnel traffic).
        t0 = time.time()
        args = list(self.zeros_in())
        args += [self._rep(z) for z in self.extra_inputs.values()]
        args.append(self.zeros_y())
        outs = self.sharded(*args)
        jax.block_until_ready(outs)
        _tlog(f"warmup jit+compile: {time.time()-t0:.2f} s")

    def _rep(self, z):
        jax = self.jax
        big = np.concatenate([z] * NCORES, axis=0)
        return jax.device_put(big, self.sharding)

    def put_chunk(self, arrs):
        """arrs: (NCORES, CR, cols) np array; puts per-core shard async."""
        jax = self.jax
        return [jax.device_put(arrs[r], self.devices[r]) for r in range(NCORES)]

    def run_chunks(self, sx_chunks, sg_chunks):
        jax = self.jax
        t0 = time.time()
        args = []
        for q in range(NCH):
            args.append(
                jax.make_array_from_single_device_arrays(
                    (NCORES * CR, FC * XW), self.sharding, sx_chunks[q]
                )
            )
        for q in range(NCH):
            args.append(
                jax.make_array_from_single_device_arrays(
                    (NCORES * CR, FC * GW), self.sharding, sg_chunks[q]
                )
            )
        args += [self._rep(z) for z in self.extra_inputs.values()]
        args.append(self.zeros_y())
        outs = self.sharded(*args)
        jax.block_until_ready(outs)
        t1 = time.time()
        shards = sorted(
            outs[0].addressable_shards, key=lambda s: s.index[0].start or 0
        )
        datas = [s.data for s in shards]
        for d in datas:
            d.copy_to_host_async()
        t2 = time.time()
        _tlog(f"  put-wait+exec: {t1-t0:.2f} s  fetch-issue: {t2-t1:.2f} s")
        return datas


def _get_runner():
    global _runner
    if _runner is None:
        last = None
        for attempt in range(3):
            try:
                _runner = _Runner()
                break
            except Exception as e:  # e.g. transient axon "mesh desynced"
                last = e
                _tlog(f"runner init attempt {attempt} failed: {e!r}")
                time.sleep(20)
        else:
            raise last
    return _runner


def kernel(x: np.ndarray, weight: np.ndarray, bias: np.ndarray) -> np.ndarray:
    x = np.ascontiguousarray(x, np.float32)
    weight = np.ascontiguousarray(weight, np.float32)
    bias = np.asarray(bias, np.float32)

    runner = _get_runner()
    t0 = time.time()
    Xf = sfft.rfft(x, n=N, axis=-1)  # (B, C, 4097) complex64
    x4096 = np.ascontiguousarray(Xf[:, :, 4096].real.T)  # (C, B)
    w4096 = np.empty((C, O), np.float32)

    sx_chunks, sg_chunks = [], []
    xrow = np.empty((4096, XW), BF16)
    grow = np.empty((4096, GW), BF16)
    WcT = np.empty((4097, O), np.complex64)
    for q in range(NCH):
        # fresh buffers per chunk: the async device_put may still be
        # streaming from the previous chunk's memory
        xch = np.empty((NCORES, CR, FC * XW), BF16)
        gch = np.empty((NCORES, CR, FC * GW), BF16)
        for cc in range(CR):
            c = q * CR + cc
            A = np.ascontiguousarray(Xf[:, c, :4096].T)  # (4096, 32) c64
            xrow[:] = A.view(np.float32)                 # (4096, 64) re/im
            xch[:, cc, :] = xrow.reshape(NCORES, FC * XW)
            Wc = sfft.rfft(weight[:, c, :], n=N, axis=-1)  # (128, 4097) c64
            w4096[c] = Wc[:, 4096].real
            WcT[:] = Wc.T
            Wv = WcT.view(np.float32)  # (4097, 256) re/im interleaved over o
            grow[:, :O] = Wv[:4096, 0::2]
            grow[:, O:] = Wv[:4096, 1::2]
            gch[:, cc, :] = grow.reshape(NCORES, FC * GW)
        # chunk q complete for all cores: ship it while packing chunk q+1
        sx_chunks.append(runner.put_chunk(xch))
        sg_chunks.append(runner.put_chunk(gch))
    t1 = time.time()
    _tlog(f"pack+put: {t1-t0:.2f} s")

    datas = runner.run_chunks(sx_chunks, sg_chunks)
    t2 = time.time()

    Yfull = np.empty((O, F, B), np.complex64)
    Yv = Yfull.view(np.float32).reshape(O, F, 2 * B)
    for r in range(NCORES):
        Yv[:, FC * r : FC * (r + 1), :] = np.asarray(datas[r]).reshape(O, FC, YW)
    Yfull[:, 4096, :] = (w4096.T @ x4096).astype(np.complex64)
    t3 = time.time()
    _tlog(f"run+fetch: {t2-t1:.2f} s  assemble: {t3-t2:.2f} s")
    yt = sfft.irfft(Yfull, n=N, axis=1)  # (O, 8192, B) f32
    out = np.empty((B, O, L), np.float32)
    out[:, :, : L - 1] = yt[:, 4097:8192, :].transpose(2, 0, 1)
    out[:, :, L - 1] = yt[:, 0, :].T
    out += bias[None, :, None]
    _tlog(f"post: {time.time()-t3:.2f} s")
    return out


if not os.environ.get("KV_NO_EAGER"):
    try:
        _get_runner()
    except Exception as e:  # defer to first call if devices unavailable now
        _tlog(f"eager init failed ({e!r}); will retry lazily")
        _runner = None


# revision 5
# speedup vs baseline: 40.0779x; 1.1241x over previous
"""Causal FFT-conv (B=32, Cin=Cout=128, L=K=4096) on 8 trn2 NeuronCores.

out = conv1d(x, w, causal) computed as
  out = irfft( rfft(x,8192) . conj(rfft(w,8192)) )[(l+4097) mod 8192], l<4096
(no explicit padding: the reference's pads reduce to a circular output shift).

Frequency contraction on-device, sharded over bins (512/core; bin 4096 on
host). Per bin f, with V = complex-interleaved X over batch and Wi negated
once per tile (block-strided DVE op; elementwise stride-2 DVE is
pathologically slow on first exec, but stride-2 PE matmul APs are fine):
  Y[o, 2k]   = sum_c Wr[c,o] Xr[c,bk] + Wi[c,o] Xi[c,bk]      (matmuls 1+2a)
  Y[o, 2k+1] = sum_c Wr[c,o] Xi[c,bk] - Wi[c,o] Xr[c,bk]      (matmuls 1+2b)
so Y lands directly in complex64 memory layout (no host de-interleave).

Inputs are split into 8 row-chunks along C (xk0..7 / gk0..7); the host
packs c-major and device_puts each chunk as soon as its rows are complete,
overlapping ~7/8 of the tunnel upload with packing. Output assembly is
folded into the pipelined per-shard fetch.
"""

import os
import sys
import time

sys.path.insert(0, "/opt/trn_rl_repo")

import numpy as np
import scipy.fft as sfft
import ml_dtypes

BF16 = ml_dtypes.bfloat16

B, C, O, L, K = 32, 128, 128, 4096, 4096
N = 8192
F = N // 2 + 1      # 4097
NCORES = 8
FC = 512            # frequency bins per core on device (8*512 = 4096)
FB = 32             # bins per For_i iteration
NITER = FC // FB    # 16
XW = 2 * B          # 64 cols/bin in xk: V only (re/im interleaved over b)
GW = 2 * O          # 256 cols/bin in gk: [Wr(128) | Wi(128)]
YW = 2 * B          # 64 cols/bin in y: interleaved (re,im) over b
NCH = 8             # C-chunks for upload/pack overlap
CR = C // NCH       # 32 rows per chunk

last_exec_ns = None
_runner = None

_DEV_TIMING = bool(os.environ.get("KV_TIMING"))


def _tlog(msg):
    if _DEV_TIMING:
        print(f"[kv] {msg}", file=sys.stderr, flush=True)


def _build_bass():
    from concourse import bacc, mybir
    from concourse.bass import ts
    from concourse.tile import TileContext

    dt = mybir.dt.float32
    dtb = mybir.dt.bfloat16
    nc = bacc.Bacc(None, target_bir_lowering=False)
    xks = [
        nc.dram_tensor(f"xk{q}", [CR, FC * XW], dtb, kind="ExternalInput")
        for q in range(NCH)
    ]
    gks = [
        nc.dram_tensor(f"gk{q}", [CR, FC * GW], dtb, kind="ExternalInput")
        for q in range(NCH)
    ]
    y = nc.dram_tensor("y", [O, FC * YW], dtb, kind="ExternalOutput")

    with TileContext(nc) as tc:
        with (
            tc.tile_pool(name="xin", bufs=3) as xpool,
            tc.tile_pool(name="gin", bufs=3) as gpool,
            tc.tile_pool(name="yout", bufs=3) as ypool,
            tc.tile_pool(name="ps", bufs=8, space="PSUM") as pspool,
        ):
            with tc.For_i(0, NITER, 1) as it:
                xt = xpool.tile([C, FB * XW], dtb, tag="x")
                gt = gpool.tile([C, FB * GW], dtb, tag="g")
                for q in range(NCH):
                    nc.gpsimd.dma_start(
                        out=xt[CR * q : CR * (q + 1), :],
                        in_=xks[q][:, ts(it, FB * XW)],
                    )
                    nc.gpsimd.dma_start(
                        out=gt[CR * q : CR * (q + 1), :],
                        in_=gks[q][:, ts(it, FB * GW)],
                    )
                # negated Wi blocks (block-strided 3D AP, 128-contig runs)
                nwt = gpool.tile([C, FB * O], dtb, tag="nw")
                nc.vector.tensor_scalar_mul(
                    nwt.rearrange("c (f o) -> c f o", o=O),
                    gt.rearrange("c (f w) -> c f w", w=GW)[:, :, O : 2 * O],
                    -1.0,
                )
                yo = ypool.tile([O, FB * YW], dtb, tag="y")
                for g in range(FB // 8):
                    ps = pspool.tile([O, 8 * YW], dt, tag="ps")  # one PSUM bank
                    for j in range(8):
                        k = g * 8 + j
                        V = xt[:, k * XW : (k + 1) * XW]
                        Vodd = xt[:, k * XW + 1 : (k + 1) * XW : 2]   # Xi cols
                        Vev = xt[:, k * XW : (k + 1) * XW : 2]        # Xr cols
                        Wr = gt[:, k * GW : k * GW + O]
                        Wi = gt[:, k * GW + O : k * GW + 2 * O]
                        nWi = nwt[:, k * O : (k + 1) * O]
                        o_sl = ps[:, j * YW : (j + 1) * YW]
                        o_ev = ps[:, j * YW : (j + 1) * YW : 2]
                        o_od = ps[:, j * YW + 1 : (j + 1) * YW : 2]
                        nc.tensor.matmul(o_sl, Wr, V, start=(j == 0), stop=False)
                        nc.tensor.matmul(o_ev, Wi, Vodd, start=False, stop=False)
                        nc.tensor.matmul(o_od, nWi, Vev, start=False, stop=(j == 7))
                    nc.vector.tensor_copy(yo[:, g * 8 * YW : (g + 1) * 8 * YW], ps)
                nc.gpsimd.dma_start(out=y[:, ts(it, FB * YW)], in_=yo)
    nc.compile()
    return nc


class _Runner:
    """Builds the jit(shard_map(bass_exec)) once; reuses it per call."""

    def __init__(self):
        import jax
        import jax.numpy as jnp
        from jax.sharding import Mesh, NamedSharding, PartitionSpec
        from jax.experimental.shard_map import shard_map
        from concourse import bass2jax, mybir

        t0 = time.time()
        self.jax = jax
        nc = _build_bass()
        self.nc = nc
        _tlog(f"build_bass: {time.time()-t0:.2f} s")

        bass2jax.install_neuronx_cc_hook()

        partition_name = (
            nc.partition_id_tensor.name if nc.partition_id_tensor else None
        )
        self.chunk_names = [f"xk{q}" for q in range(NCH)] + [
            f"gk{q}" for q in range(NCH)
        ]
        in_names, out_names, out_avals = [], [], []
        self.extra_inputs = {}  # name -> np zeros (e.g. dbg_addr)
        for alloc in nc.m.functions[0].allocations:
            if not isinstance(alloc, mybir.MemoryLocationSet):
                continue
            name = alloc.memorylocations[0].name
            if alloc.kind == "ExternalInput":
                if name != partition_name:
                    in_names.append(name)
                    if name not in self.chunk_names:
                        if nc.dbg_addr is not None and name == nc.dbg_addr.name:
                            self.extra_inputs[name] = np.zeros((1, 2), np.uint32)
                        else:
                            self.extra_inputs[name] = np.zeros(
                                tuple(alloc.tensor_shape), mybir.dt.np(alloc.dtype)
                            )
            elif alloc.kind == "ExternalOutput":
                out_names.append(name)
                out_avals.append(
                    jax.core.ShapedArray(
                        tuple(alloc.tensor_shape), mybir.dt.np(alloc.dtype)
                    )
                )
        assert out_names == ["y"], out_names
        assert in_names[: 2 * NCH] == self.chunk_names, in_names
        n_params = len(in_names)
        all_in = list(in_names) + list(out_names)
        if partition_name is not None:
            all_in.append(partition_name)
        donate = tuple(range(n_params, n_params + len(out_names)))

        def _body(*args):
            operands = list(args)
            if partition_name is not None:
                operands.append(bass2jax.partition_id_tensor())
            outs = bass2jax._bass_exec_p.bind(
                *operands,
                out_avals=tuple(out_avals),
                in_names=tuple(all_in),
                out_names=tuple(out_names),
                lowering_input_output_aliases=(),
                sim_require_finite=True,
                sim_require_nnan=True,
                nc=nc,
            )
            return tuple(outs)

        devices = jax.devices()[:NCORES]
        assert len(devices) == NCORES
        self.devices = devices
        mesh = Mesh(np.asarray(devices), ("core",))
        self.sharding = NamedSharding(mesh, PartitionSpec("core"))
        in_specs = (PartitionSpec("core"),) * (n_params + len(out_names))
        out_specs = (PartitionSpec("core"),) * len(out_names)
        self.sharded = jax.jit(
            shard_map(
                _body,
                mesh=mesh,
                in_specs=in_specs,
                out_specs=out_specs,
                check_rep=False,
            ),
            donate_argnums=donate,
            keep_unused=True,
        )
        sh = self.sharding

        def _zeros_all():
            xs = tuple(
                jnp.zeros((NCORES * CR, FC * XW), jnp.bfloat16) for _ in range(NCH)
            )
            gs = tuple(
                jnp.zeros((NCORES * CR, FC * GW), jnp.bfloat16) for _ in range(NCH)
            )
            return xs + gs

        self.zeros_in = jax.jit(_zeros_all, out_shardings=(sh,) * (2 * NCH))
        self.zeros_y = jax.jit(
            lambda: jnp.zeros((NCORES * O, FC * YW), jnp.bfloat16), out_shardings=sh
        )
        # Warm up: trace + neuronxcc compile + device init with
        # device-resident zeros (no tunnel traffic).
        t0 = time.time()
        args = list(self.zeros_in())
        args += [self._rep(z) for z in self.extra_inputs.values()]
        args.append(self.zeros_y())
        outs = self.sharded(*args)
        jax.block_until_ready(outs)
        _tlog(f"warmup jit+compile: {time.time()-t0:.2f} s")

    def _rep(self, z):
        jax = self.jax
        big = np.concatenate([z] * NCORES, axis=0)
        return jax.device_put(big, self.sharding)

    def put_chunk(self, arrs):
        """arrs: (NCORES, CR, cols) np array; puts per-core shard async."""
        jax = self.jax
        return [jax.device_put(arrs[r], self.devices[r]) for r in range(NCORES)]

    def run_chunks(self, sx_chunks, sg_chunks):
        jax = self.jax
        t0 = time.time()
        args = []
        for q in range(NCH):
            args.append(
                jax.make_array_from_single_device_arrays(
                    (NCORES * CR, FC * XW), self.sharding, sx_chunks[q]
                )
            )
        for q in range(NCH):
            args.append(
                jax.make_array_from_single_device_arrays(
                    (NCORES * CR, FC * GW), self.sharding, sg_chunks[q]
                )
            )
        args += [self._rep(z) for z in self.extra_inputs.values()]
        args.append(self.zeros_y())
        outs = self.sharded(*args)
        jax.block_until_ready(outs)
        t1 = time.time()
        shards = sorted(
            outs[0].addressable_shards, key=lambda s: s.index[0].start or 0
        )
        datas = [s.data for s in shards]
        for d in datas:
            d.copy_to_host_async()
        t2 = time.time()
        _tlog(f"  put-wait+exec: {t1-t0:.2f} s  fetch-issue: {t2-t1:.2f} s")
        return datas


def _get_runner():
    global _runner
    if _runner is None:
        last = None
        for attempt in range(3):
            try:
                _runner = _Runner()
                break
            except Exception as e:  # e.g. transient axon "mesh desynced"
                last = e
                _tlog(f"runner init attempt {attempt} failed: {e!r}")
                time.sleep(20)
        else:
            raise last
    return _runner


def kernel(x: np.ndarray, weight: np.ndarray, bias: np.ndarray) -> np.ndarray:
    x = np.ascontiguousarray(x, np.float32)
    weight = np.ascontiguousarray(weight, np.float32)
    bias = np.asarray(bias, np.float32)

    runner = _get_runner()
    t0 = time.time()
    Xf = sfft.rfft(x, n=N, axis=-1)  # (B, C, 4097) complex64
    x4096 = np.ascontiguousarray(Xf[:, :, 4096].real.T)  # (C, B)
    w4096 = np.empty((C, O), np.float32)

    sx_chunks, sg_chunks = [], []
    xrow = np.empty((4096, XW), BF16)
    grow = np.empty((4096, GW), BF16)
    WcT = np.empty((4097, O), np.complex64)
    for q in range(NCH):
        # fresh buffers per chunk: the async device_put may still be
        # streaming from the previous chunk's memory
        xch = np.empty((NCORES, CR, FC * XW), BF16)
        gch = np.empty((NCORES, CR, FC * GW), BF16)
        for cc in range(CR):
            c = q * CR + cc
            A = np.ascontiguousarray(Xf[:, c, :4096].T)  # (4096, 32) c64
            xrow[:] = A.view(np.float32)                 # (4096, 64) re/im
            xch[:, cc, :] = xrow.reshape(NCORES, FC * XW)
            Wc = sfft.rfft(weight[:, c, :], n=N, axis=-1)  # (128, 4097) c64
            w4096[c] = Wc[:, 4096].real
            WcT[:] = Wc.T
            Wv = WcT.view(np.float32)  # (4097, 256) re/im interleaved over o
            grow[:, :O] = Wv[:4096, 0::2]
            grow[:, O:] = Wv[:4096, 1::2]
            gch[:, cc, :] = grow.reshape(NCORES, FC * GW)
        # chunk q complete for all cores: ship it while packing chunk q+1
        sx_chunks.append(runner.put_chunk(xch))
        sg_chunks.append(runner.put_chunk(gch))
    t1 = time.time()
    _tlog(f"pack+put: {t1-t0:.2f} s")

    datas = runner.run_chunks(sx_chunks, sg_chunks)
    t2 = time.time()

    Yfull = np.empty((O, F, B), np.complex64)
    Yv = Yfull.view(np.float32).reshape(O, F, 2 * B)
    for r in range(NCORES):
        Yv[:, FC * r : FC * (r + 1), :] = np.asarray(datas[r]).reshape(O, FC, YW)
    Yfull[:, 4096, :] = (w4096.T @ x4096).astype(np.complex64)
    t3 = time.time()
    _tlog(f"run+fetch: {t2-t1:.2f} s  assemble: {t3-t2:.2f} s")
    yt = sfft.irfft(Yfull, n=N, axis=1)  # (O, 8192, B) f32
    out = np.empty((B, O, L), np.float32)
    out[:, :, : L - 1] = yt[:, 4097:8192, :].transpose(2, 0, 1)
    out[:, :, L - 1] = yt[:, 0, :].T
    out += bias[None, :, None]
    _tlog(f"post: {time.time()-t3:.2f} s")
    return out


if not os.environ.get("KV_NO_EAGER"):
    try:
        _get_runner()
    except Exception as e:  # defer to first call if devices unavailable now
        _tlog(f"eager init failed ({e!r}); will retry lazily")
        _runner = None
